# revision 1
# baseline (speedup 1.0000x reference)
"""Bidirectional attention kernel for Trainium2 (8 NeuronCores, data-parallel over batch).

Math per example (B=32, L1=L2=512, D=1024, fp32):
    sim = v1 @ v2^T                                  [512, 512]
    attn1 = softmax_j(sim + v2maskbias)              (mask v2 cols)
    attn2 = softmax_i(sim + v1maskbias)              (mask v1 rows)
    out1  = (attn1 @ v2) zeroed at v1-masked rows    [512, 1024]
    out2  = (attn2^T @ v1) zeroed at v2-masked rows  [512, 1024]

Device strategy (4 examples per core):
  - v1/v2 transposed on-chip via PE identity-transposes (fp32 DMA transpose
    doesn't exist); sim computed with float32r matmuls (full PE rate at N=512).
  - Negated masked logits kept so exp() runs as activation(scale=-1,
    bias=min-accumulator) with zero extra negation ops; row-sums come free via
    the activation accumulator; 1/sum and final mask-zeroing fold into the
    PSUM->SBUF output copy as a per-partition activation scale.
  - Each attn's softmax axis equals its matmul contraction axis, so the exp'd
    numerators are PE-transposed into lhsT layout ([j,i] for attn1, [i,j] for
    attn2); stats stay per-partition in the layout where they're consumed.
"""

import numpy as np

B, L, D = 32, 512, 1024
NCORES = 8
EPC = B // NCORES  # examples per core
NB = L // 128      # 128-row blocks per L
ND = D // 128      # 128-col chunks per D
NDC = D // 512     # 512-col chunks per D

_CACHE = {}
LAST_RESULTS = None


def _build_nc():
    from contextlib import ExitStack
    import concourse.bacc as bacc
    import concourse.tile as tile
    import concourse.mybir as mybir
    import concourse.bass_isa as bass_isa

    f32 = mybir.dt.float32
    f32r = mybir.dt.float32r
    EXP = mybir.ActivationFunctionType.Exp
    COPY = mybir.ActivationFunctionType.Copy
    ADD = mybir.AluOpType.add
    SUB = mybir.AluOpType.subtract
    MIN = mybir.AluOpType.min
    MUL = mybir.AluOpType.mult
    AXX = mybir.AxisListType.X

    nc = bacc.Bacc("TRN2", target_bir_lowering=False, debug=False, num_devices=NCORES)
    v1d = nc.dram_tensor("v1", [EPC * L, D], f32r, kind="ExternalInput")
    v2d = nc.dram_tensor("v2", [EPC * L, D], f32r, kind="ExternalInput")
    v2td = nc.dram_tensor("v2t", [EPC * D, L], f32r, kind="ExternalInput")
    b2d = nc.dram_tensor("b2r", [EPC * 128, L], f32, kind="ExternalInput")
    cmd = nc.dram_tensor("cm", [128, 2 * EPC * NB], f32, kind="ExternalInput")
    idd = nc.dram_tensor("idn", [128, 128], f32, kind="ExternalInput")
    bcd = nc.dram_tensor("bcol", [128, EPC * NB], f32, kind="ExternalInput")
    ond = nc.dram_tensor("onesr", [128, 2], f32r, kind="ExternalInput")
    o1d = nc.dram_tensor("o1", [EPC * L, D], f32, kind="ExternalOutput")
    o2d = nc.dram_tensor("o2", [EPC * L, D], f32, kind="ExternalOutput")
    v1a, v2a, o1a, o2a = v1d.ap(), v2d.ap(), o1d.ap(), o2d.ap()
    v2ta = v2td.ap()

    with ExitStack() as ctx:
        tc = ctx.enter_context(tile.TileContext(nc))
        const = ctx.enter_context(tc.tile_pool(name="const", bufs=1))
        pv = ctx.enter_context(tc.tile_pool(name="pv", bufs=1))
        pvt = ctx.enter_context(tc.tile_pool(name="pvt", bufs=1))
        pe_ = ctx.enter_context(tc.tile_pool(name="pe", bufs=1))
        pst = ctx.enter_context(tc.tile_pool(name="pst", bufs=1))
        pbb = ctx.enter_context(tc.tile_pool(name="pbb", bufs=1))
        pav = ctx.enter_context(tc.tile_pool(name="pav", bufs=1))
        pps = ctx.enter_context(tc.tile_pool(name="pps", bufs=1, space="PSUM"))

        ident = const.tile([128, 128], f32)
        nc.sync.dma_start(out=ident, in_=idd.ap())
        cms = const.tile([128, 2 * EPC * NB], f32)
        nc.sync.dma_start(out=cms, in_=cmd.ap())
        bcs = const.tile([128, EPC * NB], f32)
        nc.sync.dma_start(out=bcs, in_=bcd.ap())
        onesr = const.tile([128, 2], f32r)
        nc.sync.dma_start(out=onesr, in_=ond.ap())

        def trans(ps_slice, src_slice):
            if src_slice.dtype == f32r:
                src_slice = src_slice.bitcast(f32)
            nc.tensor.transpose(ps_slice, src_slice, ident)

        for e in range(EPC):
            r0 = e * L
            v1sb = [pv.tile([128, D], f32r, tag="v1", bufs=8, name=f"v1sb_{e}_{b}") for b in range(NB)]
            v2sb = [pv.tile([128, D], f32r, tag="v2", bufs=8, name=f"v2sb_{e}_{b}") for b in range(NB)]
            for b in range(NB):
                nc.sync.dma_start(out=v1sb[b], in_=v1a[r0 + b * 128 : r0 + (b + 1) * 128, :])
            b2bc = pbb.tile([128, L], f32, tag="b2", bufs=2)
            nc.sync.dma_start(out=b2bc, in_=b2d.ap()[e * 128 : (e + 1) * 128, :])

            # ---- v2T loaded pre-transposed from host; v1T via PE transposes ----
            v2T = []
            for c in range(ND):
                t = pvt.tile([128, 512], f32r, tag="v2T", bufs=16, name=f"v2T_{e}_{c}")
                nc.sync.dma_start(out=t, in_=v2ta[e * D + c * 128 : e * D + (c + 1) * 128, :])
                v2T.append(t)
            for b in range(NB):
                nc.sync.dma_start(out=v2sb[b], in_=v2a[r0 + b * 128 : r0 + (b + 1) * 128, :])
            v1T = []
            for c in range(ND):
                ps = pps.tile([128, 512], f32, tag="pti", bufs=2, name=f"ptr_{e}_v1T_{c}")
                for b in range(NB):
                    trans(ps[:, b * 128 : (b + 1) * 128], v1sb[b][:, c * 128 : (c + 1) * 128])
                t = pvt.tile([128, 512], f32r, tag="v1T", bufs=8, name=f"v1T_{e}_{c}")
                nc.vector.tensor_copy(t, ps)
                v1T.append(t)

            # ---- sim (ij layout); mk = sim + b2row; global bound gm ----
            s1t = pst.tile([128, NB], f32, tag="s1t", bufs=4, name=f"s1t_{e}")
            r1t = pst.tile([128, NB], f32, tag="r1t", bufs=4, name=f"r1t_{e}")
            sc1t = pst.tile([128, NB], f32, tag="sc1t", bufs=4, name=f"sc1t_{e}")
            m1t = pst.tile([128, NB], f32, tag="m1t", bufs=4, name=f"m1t_{e}")
            mk_ij, e1ij = [], []
            for ib in range(NB):
                ps = pps.tile([128, 512], f32, tag="sim", bufs=2)
                for c in range(ND):
                    nc.tensor.matmul(
                        ps,
                        v1T[c][:, ib * 128 : (ib + 1) * 128],
                        v2T[c],
                        start=(c == 0),
                        stop=(c == ND - 1),
                    )
                mk = pe_.tile([128, 512], f32, tag="mk", bufs=4)
                nc.vector.tensor_add(mk, ps, b2bc)
                nc.vector.reduce_max(m1t[:, ib : ib + 1], mk, axis=AXX)
                m1nb = pst.tile([128, 1], f32, tag="m1nb", bufs=8, name=f"m1nb_{e}_{ib}")
                nc.vector.tensor_scalar_mul(m1nb, m1t[:, ib : ib + 1], -1.0)
                e1 = pe_.tile([128, 512], f32, tag="e1ij", bufs=4, name=f"e1_{e}_{ib}")
                nc.scalar.activation(out=e1, in_=mk, func=EXP, bias=m1nb, scale=1.0,
                                     accum_out=s1t[:, ib : ib + 1])
                e1ij.append(e1)
                mk_ij.append(mk)
            # gm = max over all rows/blocks (upper bound for both softmaxes)
            gmx = pst.tile([128, 1], f32, tag="gmx", bufs=4, name=f"gmx_{e}")
            nc.vector.reduce_max(gmx, m1t, axis=AXX)
            gmr = pst.tile([128, 1], f32, tag="gmr", bufs=4, name=f"gmr_{e}")
            nc.gpsimd.partition_all_reduce(gmr, gmx, 128, bass_isa.ReduceOp.max)
            # bias = 60 - gm: keeps per-column softmax numerators in normal
            # fp32 range (safe for column maxes up to ~147 below gm) while
            # sums stay <= 512*e^60, far from overflow.
            gmn = pst.tile([128, 1], f32, tag="gmn", bufs=4, name=f"gmn_{e}")
            nc.vector.tensor_scalar(gmn, gmr, -1.0, 60.0, op0=MUL, op1=ADD)
            # comb2 = b1col - gm  (per-partition bias for e2)
            comb2 = pst.tile([128, NB], f32, tag="comb2", bufs=4, name=f"comb2_{e}")
            nc.vector.tensor_scalar_add(comb2, bcs[:, e * NB : e * NB + NB], gmn)
            nc.vector.reciprocal(out=r1t, in_=s1t)
            nc.vector.tensor_mul(sc1t, r1t, cms[:, e * NB : e * NB + NB])
            # e2_ij = exp(mk + b1col - gm)  (b2row term cancels per-column)
            e2ij = []
            for ib in range(NB):
                e2 = pe_.tile([128, 512], f32r, tag="e2ij", bufs=5, name=f"e2ij_{e}_{ib}")
                nc.scalar.activation(out=e2, in_=mk_ij[ib], func=EXP,
                                     bias=comb2[:, ib : ib + 1], scale=1.0)
                e2ij.append(e2)
            # s2 columns via ones-matmuls: s2col[jb] = sum_i e2ij[:, jb-block]
            pss = pps.tile([128, 2 * NB], f32, tag="att", bufs=2, name=f"pss_{e}")
            for jb in range(NB):
                for ib in range(NB):
                    nc.tensor.matmul(pss[:, 2 * jb : 2 * jb + 2],
                                     e2ij[ib][:, jb * 128 : (jb + 1) * 128], onesr,
                                     start=(ib == 0), stop=(ib == NB - 1))
            s2t = pst.tile([128, NB], f32, tag="s2t", bufs=4, name=f"s2t_{e}")
            nc.vector.tensor_scalar_add(s2t, pss[:, 0 : 2 * NB : 2], 1.0e-36)
            r2t = pst.tile([128, NB], f32, tag="r2t", bufs=4, name=f"r2t_{e}")
            nc.vector.reciprocal(out=r2t, in_=s2t)
            sc2t = pst.tile([128, NB], f32, tag="sc2t", bufs=4, name=f"sc2t_{e}")
            nc.vector.tensor_mul(sc2t, r2t, cms[:, EPC * NB + e * NB : EPC * NB + e * NB + NB])

            # ---- transpose e1 numerators into [j,i] lhsT layout ----
            e1ji = []
            for jb in range(NB):
                ps = pps.tile([128, 512], f32, tag="pte", bufs=2, name=f"pt1_{e}_{jb}")
                for ib in range(NB):
                    trans(ps[:, ib * 128 : (ib + 1) * 128], e1ij[ib][:, jb * 128 : (jb + 1) * 128])
                t = pe_.tile([128, 512], f32r, tag="e1ji", bufs=5, name=f"e1ji_{e}_{jb}")
                nc.scalar.copy(t, ps)
                e1ji.append(t)

            # ---- attends: out1[i,d] = sum_j e1[j,i] v2[j,d] / s1, out2 sym ----
            for ib in range(NB):
                av = pav.tile([128, D], f32, tag="av1", bufs=3)
                for dc in range(NDC):
                    ps = pps.tile([128, 512], f32, tag="att", bufs=2)
                    for jb in range(NB):
                        nc.tensor.matmul(
                            ps,
                            e1ji[jb][:, ib * 128 : (ib + 1) * 128],
                            v2sb[jb][:, dc * 512 : (dc + 1) * 512],
                            start=(jb == 0),
                            stop=(jb == NB - 1),
                        )
                    if dc == 0:
                        nc.scalar.activation(out=av[:, dc * 512 : (dc + 1) * 512], in_=ps, func=COPY, scale=sc1t[:, ib : ib + 1])
                    else:
                        nc.vector.tensor_scalar_mul(av[:, dc * 512 : (dc + 1) * 512], ps, sc1t[:, ib : ib + 1])
                nc.scalar.dma_start(out=o1a[r0 + ib * 128 : r0 + (ib + 1) * 128, :], in_=av)
            for jb in range(NB):
                av = pav.tile([128, D], f32, tag="av2", bufs=3)
                for dc in range(NDC):
                    ps = pps.tile([128, 512], f32, tag="att", bufs=2)
                    for ib in range(NB):
                        nc.tensor.matmul(
                            ps,
                            e2ij[ib][:, jb * 128 : (jb + 1) * 128],
                            v1sb[ib][:, dc * 512 : (dc + 1) * 512],
                            start=(ib == 0),
                            stop=(ib == NB - 1),
                        )
                    if dc == 0:
                        nc.scalar.activation(out=av[:, dc * 512 : (dc + 1) * 512], in_=ps, func=COPY, scale=sc2t[:, jb : jb + 1])
                    else:
                        nc.vector.tensor_scalar_mul(av[:, dc * 512 : (dc + 1) * 512], ps, sc2t[:, jb : jb + 1])
                nc.scalar.dma_start(out=o2a[r0 + jb * 128 : r0 + (jb + 1) * 128, :], in_=av)

    nc.compile()
    return nc


def get_nc():
    if "nc" not in _CACHE:
        _CACHE["nc"] = _build_nc()
    return _CACHE["nc"]


def _host_prep(v1, v2, v1_mask, v2_mask):
    """Build per-core input maps from full inputs."""
    v1 = np.asarray(v1, dtype=np.float32)
    v2 = np.asarray(v2, dtype=np.float32)
    v1_mask = np.asarray(v1_mask).astype(bool)
    v2_mask = np.asarray(v2_mask).astype(bool)
    in_maps = []
    for k in range(NCORES):
        sl = slice(EPC * k, EPC * (k + 1))
        m1 = v1_mask[sl]
        m2 = v2_mask[sl]
        b1 = np.where(m1, np.float32(-1e30), np.float32(0.0)).astype(np.float32)
        b2 = np.where(m2, np.float32(-1e30), np.float32(0.0)).astype(np.float32)
        bcol = np.ascontiguousarray(b1.reshape(EPC, NB, 128).transpose(2, 0, 1).reshape(128, EPC * NB))
        b2 = np.repeat(b2[:, None, :], 128, axis=1).reshape(EPC * 128, L)
        # keep-columns: cm[p, e*NB+b] = 1-v1_mask[e, b*128+p]; second half for v2
        k1 = (~m1).astype(np.float32).reshape(EPC, NB, 128).transpose(2, 0, 1).reshape(128, EPC * NB)
        k2 = (~m2).astype(np.float32).reshape(EPC, NB, 128).transpose(2, 0, 1).reshape(128, EPC * NB)
        in_maps.append(
            {
                "v1": np.ascontiguousarray(v1[sl].reshape(EPC * L, D)),
                "v2": np.ascontiguousarray(v2[sl].reshape(EPC * L, D)),
                "v2t": np.ascontiguousarray(v2[sl].transpose(0, 2, 1).reshape(EPC * D, L)),
                "b2r": np.ascontiguousarray(b2),
                "bcol": bcol,
                "onesr": np.ones((128, 2), np.float32),
                "cm": np.ascontiguousarray(np.concatenate([k1, k2], axis=1)),
                "idn": np.eye(128, dtype=np.float32),
            }
        )
    return in_maps


def kernel(v1, v2, v1_mask, v2_mask):
    global LAST_RESULTS
    from concourse.bass_utils import run_bass_kernel_spmd

    nc = get_nc()
    in_maps = _host_prep(v1, v2, v1_mask, v2_mask)
    res = run_bass_kernel_spmd(nc, in_maps, list(range(NCORES)))
    LAST_RESULTS = res
    o1 = np.concatenate(
        [res.results[k]["o1"].reshape(EPC, L, D) for k in range(NCORES)], axis=0
    )
    o2 = np.concatenate(
        [res.results[k]["o2"].reshape(EPC, L, D) for k in range(NCORES)], axis=0
    )
    return o1, o2



# revision 3
# speedup vs baseline: 1.3344x; 1.3344x over previous
"""Bidirectional attention kernel for Trainium2 (8 NeuronCores, data-parallel over batch).

Math per example (B=32, L1=L2=512, D=1024):
    sim = v1 @ v2^T                                  [512, 512]
    attn1 = softmax_j(sim + v2maskbias)              (mask v2 cols)
    attn2 = softmax_i(sim + v1maskbias)              (mask v1 rows)
    out1  = (attn1 @ v2) zeroed at v1-masked rows    [512, 1024]
    out2  = (attn2^T @ v1) zeroed at v2-masked rows  [512, 1024]

Device strategy (4 examples per core), all-16-bit datapath:
  - sim matmul operands in fp16 (host pre-transposed v1T/v2T); PSUM fp32.
    fp16 logits keep softmax ties stable (bf16 does not: 9e-2 rel err).
  - e1 numerators fp16 (range (0,1]); e2 numerators bf16 (range up to
    e^60 from the global-max-bound trick, needs fp32 exponent range).
  - attend rhs: v2 natural fp16 (out1), v1 natural bf16 (out2, dtype
    must match e2 lhsT). Outputs stored bf16, upcast on host.
  - Softmax row-sums come free via the EXP activation accumulator; 1/sum
    and final mask-zeroing fold into the PSUM->SBUF output copy as a
    per-partition scale. Column sums for attn2 via tiny ones-matmuls.
  - 1-example software-pipeline skew: sim+stats of example e issue before
    the transpose/attend matmuls of example e-1, hiding the softmax stats
    latency (DVE->gpsimd->ACT chain) under PE work so PE never idles.
"""

import numpy as np

B, L, D = 32, 512, 1024
NCORES = 8
EPC = B // NCORES  # examples per core
NB = L // 128      # 128-row blocks per L
ND = D // 128      # 128-row chunks per D (transposed layouts)
NDC = D // 512     # 512-col halves per D

_CACHE = {}
LAST_RESULTS = None


def _build_nc():
    from contextlib import ExitStack
    import concourse.bacc as bacc
    import concourse.tile as tile
    import concourse.mybir as mybir
    import concourse.bass_isa as bass_isa

    f32 = mybir.dt.float32
    f16 = mybir.dt.float16
    bf16 = mybir.dt.bfloat16
    EXP = mybir.ActivationFunctionType.Exp
    COPY = mybir.ActivationFunctionType.Copy
    ADD = mybir.AluOpType.add
    MUL = mybir.AluOpType.mult
    AXX = mybir.AxisListType.X

    nc = bacc.Bacc("TRN2", target_bir_lowering=False, debug=False, num_devices=NCORES)
    v1td = nc.dram_tensor("v1t", [EPC * D, L], f16, kind="ExternalInput")
    v2td = nc.dram_tensor("v2t", [EPC * D, L], f16, kind="ExternalInput")
    v2nd = nc.dram_tensor("v2n", [EPC * L, D], f16, kind="ExternalInput")
    v1nd = nc.dram_tensor("v1n", [EPC * L, D], bf16, kind="ExternalInput")
    b2d = nc.dram_tensor("b2r", [EPC * 128, L], f32, kind="ExternalInput")
    cmd = nc.dram_tensor("cm", [128, 2 * EPC * NB], f32, kind="ExternalInput")
    idd = nc.dram_tensor("idh", [128, 128], f16, kind="ExternalInput")
    bcd = nc.dram_tensor("bcol", [128, EPC * NB], f32, kind="ExternalInput")
    ond = nc.dram_tensor("ones2", [128, 2], bf16, kind="ExternalInput")
    o1d = nc.dram_tensor("o1", [EPC * L, D], bf16, kind="ExternalOutput")
    o2d = nc.dram_tensor("o2", [EPC * L, D], bf16, kind="ExternalOutput")
    v1ta, v2ta, v2na, v1na = v1td.ap(), v2td.ap(), v2nd.ap(), v1nd.ap()
    o1a, o2a = o1d.ap(), o2d.ap()

    with ExitStack() as ctx:
        tc = ctx.enter_context(tile.TileContext(nc))
        const = ctx.enter_context(tc.tile_pool(name="const", bufs=1))
        pv = ctx.enter_context(tc.tile_pool(name="pv", bufs=1))
        pvt = ctx.enter_context(tc.tile_pool(name="pvt", bufs=1))
        pe_ = ctx.enter_context(tc.tile_pool(name="pe", bufs=1))
        pst = ctx.enter_context(tc.tile_pool(name="pst", bufs=1))
        pbb = ctx.enter_context(tc.tile_pool(name="pbb", bufs=1))
        pav = ctx.enter_context(tc.tile_pool(name="pav", bufs=1))
        pps = ctx.enter_context(tc.tile_pool(name="pps", bufs=1, space="PSUM"))

        ident = const.tile([128, 128], f16)
        nc.sync.dma_start(out=ident, in_=idd.ap())
        cms = const.tile([128, 2 * EPC * NB], f32)
        nc.sync.dma_start(out=cms, in_=cmd.ap())
        bcs = const.tile([128, EPC * NB], f32)
        nc.sync.dma_start(out=bcs, in_=bcd.ap())
        onesr = const.tile([128, 2], bf16)
        nc.sync.dma_start(out=onesr, in_=ond.ap())

        # per-example state carried between pipeline stages
        st = [dict() for _ in range(EPC)]

        def stage_load(e):
            s = st[e]
            r0 = e * L
            s["v1T"] = []
            s["v2T"] = []
            for c in range(ND):
                t1 = pvt.tile([128, L], f16, tag="v1T", bufs=2 * ND, name=f"v1T_{e}_{c}")
                nc.sync.dma_start(out=t1, in_=v1ta[e * D + c * 128 : e * D + (c + 1) * 128, :])
                t2 = pvt.tile([128, L], f16, tag="v2T", bufs=2 * ND, name=f"v2T_{e}_{c}")
                nc.sync.dma_start(out=t2, in_=v2ta[e * D + c * 128 : e * D + (c + 1) * 128, :])
                s["v1T"].append(t1)
                s["v2T"].append(t2)
            b2bc = pbb.tile([128, L], f32, tag="b2", bufs=2, name=f"b2bc_{e}")
            nc.sync.dma_start(out=b2bc, in_=b2d.ap()[e * 128 : (e + 1) * 128, :])
            s["b2bc"] = b2bc
            s["v2n"] = []
            s["v1n"] = []
            for b in range(NB):
                t2 = pv.tile([128, D], f16, tag="v2n", bufs=2 * NB, name=f"v2n_{e}_{b}")
                nc.sync.dma_start(out=t2, in_=v2na[r0 + b * 128 : r0 + (b + 1) * 128, :])
                t1 = pv.tile([128, D], bf16, tag="v1n", bufs=2 * NB, name=f"v1n_{e}_{b}")
                nc.sync.dma_start(out=t1, in_=v1na[r0 + b * 128 : r0 + (b + 1) * 128, :])
                s["v2n"].append(t2)
                s["v1n"].append(t1)

        def stage_sim(e):
            s = st[e]
            m1t = pst.tile([128, NB], f32, tag="m1t", bufs=2, name=f"m1t_{e}")
            s1t = pst.tile([128, NB], f32, tag="s1t", bufs=2, name=f"s1t_{e}")
            s["mk"], s["e1"], s["e2"] = [], [], []
            for ib in range(NB):
                ps = pps.tile([128, L], f32, tag="sim", bufs=2)
                for c in range(ND):
                    nc.tensor.matmul(
                        ps,
                        s["v1T"][c][:, ib * 128 : (ib + 1) * 128],
                        s["v2T"][c],
                        start=(c == 0),
                        stop=(c == ND - 1),
                    )
                mk = pe_.tile([128, L], f32, tag="mk", bufs=2 * NB, name=f"mk_{e}_{ib}")
                nc.vector.tensor_add(mk, ps, s["b2bc"])
                nc.vector.reduce_max(m1t[:, ib : ib + 1], mk, axis=AXX)
                m1nb = pst.tile([128, 1], f32, tag="m1nb", bufs=2 * NB, name=f"m1nb_{e}_{ib}")
                nc.vector.tensor_scalar_mul(m1nb, m1t[:, ib : ib + 1], -1.0)
                e1 = pe_.tile([128, L], f16, tag="e1", bufs=2 * NB, name=f"e1_{e}_{ib}")
                nc.scalar.activation(out=e1, in_=mk, func=EXP, bias=m1nb, scale=1.0,
                                     accum_out=s1t[:, ib : ib + 1])
                s["mk"].append(mk)
                s["e1"].append(e1)
            # global max bound gm for the column softmax (attn2)
            gmx = pst.tile([128, 1], f32, tag="gmx", bufs=2, name=f"gmx_{e}")
            nc.vector.reduce_max(gmx, m1t, axis=AXX)
            gmr = pst.tile([128, 1], f32, tag="gmr", bufs=2, name=f"gmr_{e}")
            nc.gpsimd.partition_all_reduce(gmr, gmx, 128, bass_isa.ReduceOp.max)
            # bias = 60 - gm keeps e2 numerators in normal fp32/bf16 range
            gmn = pst.tile([128, 1], f32, tag="gmn", bufs=2, name=f"gmn_{e}")
            nc.vector.tensor_scalar(gmn, gmr, -1.0, 60.0, op0=MUL, op1=ADD)
            comb2 = pst.tile([128, NB], f32, tag="comb2", bufs=2, name=f"comb2_{e}")
            nc.vector.tensor_scalar_add(comb2, bcs[:, e * NB : e * NB + NB], gmn)
            r1t = pst.tile([128, NB], f32, tag="r1t", bufs=2, name=f"r1t_{e}")
            nc.vector.reciprocal(out=r1t, in_=s1t)
            sc1t = pst.tile([128, NB], f32, tag="sc1t", bufs=2, name=f"sc1t_{e}")
            nc.vector.tensor_mul(sc1t, r1t, cms[:, e * NB : e * NB + NB])
            s["sc1t"] = sc1t
            # e2 = exp(mk + b1col - gm + 60); b2row term cancels per-column
            for ib in range(NB):
                e2 = pe_.tile([128, L], bf16, tag="e2", bufs=2 * NB, name=f"e2_{e}_{ib}")
                nc.scalar.activation(out=e2, in_=s["mk"][ib], func=EXP,
                                     bias=comb2[:, ib : ib + 1], scale=1.0)
                s["e2"].append(e2)

        def stage_fin(e):
            s = st[e]
            r0 = e * L
            # ---- transpose e1 numerators into [j,i] lhsT layout (fp16, 1 cyc/row)
            e1ji = []
            for jb in range(NB):
                ps = pps.tile([128, L], f16, tag="pte", bufs=2, name=f"pt1_{e}_{jb}")
                for ib in range(NB):
                    nc.tensor.transpose(
                        ps[:, ib * 128 : (ib + 1) * 128],
                        s["e1"][ib][:, jb * 128 : (jb + 1) * 128],
                        ident,
                    )
                t = pe_.tile([128, L], f16, tag="e1ji", bufs=NB + 1, name=f"e1ji_{e}_{jb}")
                nc.scalar.copy(t, ps)
                e1ji.append(t)
            # ---- out1[i,d] = sum_j e1[j,i] v2[j,d] / s1, masked rows zeroed
            for ib in range(NB):
                av = pav.tile([128, D], bf16, tag="av1", bufs=3)
                for dc in range(NDC):
                    ps = pps.tile([128, 512], f32, tag="att", bufs=2)
                    for jb in range(NB):
                        nc.tensor.matmul(
                            ps,
                            e1ji[jb][:, ib * 128 : (ib + 1) * 128],
                            s["v2n"][jb][:, dc * 512 : (dc + 1) * 512],
                            start=(jb == 0),
                            stop=(jb == NB - 1),
                        )
                    if dc == 0:
                        nc.scalar.activation(out=av[:, dc * 512 : (dc + 1) * 512],
                                             in_=ps, func=COPY,
                                             scale=s["sc1t"][:, ib : ib + 1])
                    else:
                        nc.vector.tensor_scalar_mul(av[:, dc * 512 : (dc + 1) * 512],
                                                    ps, s["sc1t"][:, ib : ib + 1])
                nc.scalar.dma_start(out=o1a[r0 + ib * 128 : r0 + (ib + 1) * 128, :], in_=av)
            # ---- s2 column sums via ones-matmuls on e2 tiles
            pss = pps.tile([128, 2 * NB], f32, tag="pss", bufs=2, name=f"pss_{e}")
            for jb in range(NB):
                for ib in range(NB):
                    nc.tensor.matmul(pss[:, 2 * jb : 2 * jb + 2],
                                     s["e2"][ib][:, jb * 128 : (jb + 1) * 128], onesr,
                                     start=(ib == 0), stop=(ib == NB - 1))
            s2t = pst.tile([128, NB], f32, tag="s2t", bufs=2, name=f"s2t_{e}")
            nc.vector.tensor_scalar_add(s2t, pss[:, 0 : 2 * NB : 2], 1.0e-36)
            r2t = pst.tile([128, NB], f32, tag="r2t", bufs=2, name=f"r2t_{e}")
            nc.vector.reciprocal(out=r2t, in_=s2t)
            sc2t = pst.tile([128, NB], f32, tag="sc2t", bufs=2, name=f"sc2t_{e}")
            nc.vector.tensor_mul(sc2t, r2t, cms[:, EPC * NB + e * NB : EPC * NB + e * NB + NB])
            # ---- out2[j,d] = sum_i e2[i,j] v1[i,d] / s2, masked rows zeroed
            for jb in range(NB):
                av = pav.tile([128, D], bf16, tag="av2", bufs=3)
                for dc in range(NDC):
                    ps = pps.tile([128, 512], f32, tag="att", bufs=2)
                    for ib in range(NB):
                        nc.tensor.matmul(
                            ps,
                            s["e2"][ib][:, jb * 128 : (jb + 1) * 128],
                            s["v1n"][ib][:, dc * 512 : (dc + 1) * 512],
                            start=(ib == 0),
                            stop=(ib == NB - 1),
                        )
                    if dc == 0:
                        nc.scalar.activation(out=av[:, dc * 512 : (dc + 1) * 512],
                                             in_=ps, func=COPY,
                                             scale=sc2t[:, jb : jb + 1])
                    else:
                        nc.vector.tensor_scalar_mul(av[:, dc * 512 : (dc + 1) * 512],
                                                    ps, sc2t[:, jb : jb + 1])
                nc.scalar.dma_start(out=o2a[r0 + jb * 128 : r0 + (jb + 1) * 128, :], in_=av)
            st[e] = {}  # release references

        stage_load(0)
        stage_sim(0)
        for e in range(1, EPC):
            stage_load(e)
            stage_sim(e)
            stage_fin(e - 1)
        stage_fin(EPC - 1)

    nc.compile()
    return nc


def get_nc():
    if "nc" not in _CACHE:
        _CACHE["nc"] = _build_nc()
    return _CACHE["nc"]


def _host_prep(v1, v2, v1_mask, v2_mask):
    """Build per-core input maps from full inputs."""
    import ml_dtypes

    bf16 = ml_dtypes.bfloat16
    v1 = np.asarray(v1, dtype=np.float32)
    v2 = np.asarray(v2, dtype=np.float32)
    v1_mask = np.asarray(v1_mask).astype(bool)
    v2_mask = np.asarray(v2_mask).astype(bool)
    in_maps = []
    for k in range(NCORES):
        sl = slice(EPC * k, EPC * (k + 1))
        v1c, v2c = v1[sl], v2[sl]
        m1 = v1_mask[sl]
        m2 = v2_mask[sl]
        b1 = np.where(m1, np.float32(-1e30), np.float32(0.0)).astype(np.float32)
        b2 = np.where(m2, np.float32(-1e30), np.float32(0.0)).astype(np.float32)
        bcol = np.ascontiguousarray(b1.reshape(EPC, NB, 128).transpose(2, 0, 1).reshape(128, EPC * NB))
        b2rep = np.repeat(b2[:, None, :], 128, axis=1).reshape(EPC * 128, L)
        # keep-columns: cm[p, e*NB+b] = 1-v1_mask[e, b*128+p]; second half for v2
        k1 = (~m1).astype(np.float32).reshape(EPC, NB, 128).transpose(2, 0, 1).reshape(128, EPC * NB)
        k2 = (~m2).astype(np.float32).reshape(EPC, NB, 128).transpose(2, 0, 1).reshape(128, EPC * NB)
        in_maps.append(
            {
                "v1t": np.ascontiguousarray(v1c.transpose(0, 2, 1).reshape(EPC * D, L)).astype(np.float16),
                "v2t": np.ascontiguousarray(v2c.transpose(0, 2, 1).reshape(EPC * D, L)).astype(np.float16),
                "v2n": np.ascontiguousarray(v2c.reshape(EPC * L, D)).astype(np.float16),
                "v1n": np.ascontiguousarray(v1c.reshape(EPC * L, D)).astype(bf16),
                "b2r": np.ascontiguousarray(b2rep),
                "bcol": bcol,
                "ones2": np.ones((128, 2), bf16),
                "cm": np.ascontiguousarray(np.concatenate([k1, k2], axis=1)),
                "idh": np.eye(128, dtype=np.float16),
            }
        )
    return in_maps


def kernel(v1, v2, v1_mask, v2_mask):
    global LAST_RESULTS
    from concourse.bass_utils import run_bass_kernel_spmd

    nc = get_nc()
    in_maps = _host_prep(v1, v2, v1_mask, v2_mask)
    res = run_bass_kernel_spmd(nc, in_maps, list(range(NCORES)))
    LAST_RESULTS = res
    o1 = np.concatenate(
        [res.results[k]["o1"].astype(np.float32).reshape(EPC, L, D) for k in range(NCORES)],
        axis=0,
    )
    o2 = np.concatenate(
        [res.results[k]["o2"].astype(np.float32).reshape(EPC, L, D) for k in range(NCORES)],
        axis=0,
    )
    return o1, o2


# revision 18
# speedup vs baseline: 1.4382x; 1.0778x over previous
"""Bidirectional attention kernel for Trainium2 (8 NeuronCores, data-parallel over batch).

Math per example (B=32, L1=L2=512, D=1024):
    sim = v1 @ v2^T                                  [512, 512]
    attn1 = softmax_j(sim + v2maskbias)              (mask v2 cols)
    attn2 = softmax_i(sim + v1maskbias)              (mask v1 rows)
    out1  = (attn1 @ v2) zeroed at v1-masked rows    [512, 1024]
    out2  = (attn2^T @ v1) zeroed at v2-masked rows  [512, 1024]

Device strategy (4 examples per core), all-16-bit datapath:
  - sim matmul operands in fp16 (host pre-transposed); PSUM fp32. fp16
    logits keep softmax ties stable (bf16 does not: 9e-2 rel err).
  - e1 numerators fp16 (range (0,1]); e2 numerators bf16 (range up to
    e^60 from the global-max-bound trick, needs fp32 exponent range).
  - attend rhs: v2 natural fp16 (out1), v1 natural bf16 (out2). Outputs
    bf16, unpacked/upcast on host.
  - Host packs each per-example tensor into a single [128, x] row-major
    region so every load/store is ONE large DMA (DMA issue instructions
    cost ~650ns each on the issuing engine and serialize).
  - e1 -> e1ji transpose on the PE (fp16 transposes run 1 cyc/row, half
    the fp32 cost); PSUM->SBUF copies balanced across ACT and DVE
    (gpsimd/Pool cannot touch PSUM); store issues on gpsimd.
  - Row softmax stats ride the EXP activation accumulator; 1/sum and
    mask-zeroing fold into PSUM->SBUF output copies (per-partition
    scale) which are spread across ACT/DVE/Pool to keep all three off
    the critical path. Column sums for attn2 via tiny ones-matmuls.
  - 1-example software-pipeline skew: sim+softmax of example e issue
    before the attend matmuls of example e-1 so the PE never waits on
    the softmax stats chain and stays at full p-state.
"""

import numpy as np

B, L, D = 32, 512, 1024
NCORES = 8
EPC = B // NCORES  # examples per core
NB = L // 128      # 128-row blocks per L
ND = D // 128      # 128-row chunks per D (transposed layouts)
NDC = D // 512     # 512-col halves per D
NDH = ND // 2      # chunks per load half

_CACHE = {}
LAST_RESULTS = None


def _build_nc():
    from contextlib import ExitStack
    import concourse.bacc as bacc
    import concourse.tile as tile
    import concourse.mybir as mybir
    import concourse.bass_isa as bass_isa

    f32 = mybir.dt.float32
    f16 = mybir.dt.float16
    bf16 = mybir.dt.bfloat16
    EXP = mybir.ActivationFunctionType.Exp
    COPY = mybir.ActivationFunctionType.Copy
    ADD = mybir.AluOpType.add
    MIN = mybir.AluOpType.min
    MAX = mybir.AluOpType.max
    AXX = mybir.AxisListType.X

    nc = bacc.Bacc("TRN2", target_bir_lowering=False, debug=False, num_devices=NCORES)
    # packed layouts: one [128, x] row-major region per example per tensor
    v1td = nc.dram_tensor("v1t", [EPC * 128, ND * L], f16, kind="ExternalInput")
    v2td = nc.dram_tensor("v2t", [EPC * 128, ND * L], f16, kind="ExternalInput")
    v2nd = nc.dram_tensor("v2n", [EPC * 128, NB * D], f16, kind="ExternalInput")
    v1nd = nc.dram_tensor("v1n", [EPC * 128, NB * D], bf16, kind="ExternalInput")
    b2d = nc.dram_tensor("b2r", [EPC * 128, L], f32, kind="ExternalInput")
    cmd = nc.dram_tensor("cm", [128, 2 * EPC * NB], f32, kind="ExternalInput")
    bcd = nc.dram_tensor("bcol", [128, EPC * NB], f32, kind="ExternalInput")
    idd = nc.dram_tensor("idh", [128, 128], f16, kind="ExternalInput")
    ond = nc.dram_tensor("ones2", [128, 2], bf16, kind="ExternalInput")
    o1d = nc.dram_tensor("o1", [EPC * 128, NB * D], bf16, kind="ExternalOutput")
    o2d = nc.dram_tensor("o2", [EPC * 128, NB * D], bf16, kind="ExternalOutput")
    v1ta, v2ta, v2na, v1na = v1td.ap(), v2td.ap(), v2nd.ap(), v1nd.ap()
    o1a, o2a = o1d.ap(), o2d.ap()

    with ExitStack() as ctx:
        tc = ctx.enter_context(tile.TileContext(nc))
        const = ctx.enter_context(tc.tile_pool(name="const", bufs=1))
        pv = ctx.enter_context(tc.tile_pool(name="pv", bufs=1))
        pvt = ctx.enter_context(tc.tile_pool(name="pvt", bufs=1))
        pe_ = ctx.enter_context(tc.tile_pool(name="pe", bufs=1))
        pst = ctx.enter_context(tc.tile_pool(name="pst", bufs=1))
        pbb = ctx.enter_context(tc.tile_pool(name="pbb", bufs=1))
        pav = ctx.enter_context(tc.tile_pool(name="pav", bufs=1))
        pps = ctx.enter_context(tc.tile_pool(name="pps", bufs=1, space="PSUM"))

        ident = const.tile([128, 128], f16)
        nc.sync.dma_start(out=ident, in_=idd.ap())
        cms = const.tile([128, 2 * EPC * NB], f32)
        nc.sync.dma_start(out=cms, in_=cmd.ap())
        bcs = const.tile([128, EPC * NB], f32)
        nc.sync.dma_start(out=bcs, in_=bcd.ap())
        onesr = const.tile([128, 2], bf16)
        nc.sync.dma_start(out=onesr, in_=ond.ap())

        st = [dict() for _ in range(EPC)]

        def stage_load(e):
            s = st[e]
            s["v1T"] = []  # two halves, each [128, 4*512] = chunks 0-3 / 4-7
            s["v2T"] = []
            for h in range(2):
                t1 = pvt.tile([128, NDH * L], f16, tag="v1T", bufs=4, name=f"v1T_{e}_{h}")
                nc.sync.dma_start(out=t1, in_=v1ta[e * 128 : (e + 1) * 128,
                                                  h * NDH * L : (h + 1) * NDH * L])
                t2 = pvt.tile([128, NDH * L], f16, tag="v2T", bufs=4, name=f"v2T_{e}_{h}")
                nc.sync.dma_start(out=t2, in_=v2ta[e * 128 : (e + 1) * 128,
                                                  h * NDH * L : (h + 1) * NDH * L])
                s["v1T"].append(t1)
                s["v2T"].append(t2)
            b2bc = pbb.tile([128, L], f32, tag="b2", bufs=2, name=f"b2bc_{e}")
            nc.sync.dma_start(out=b2bc, in_=b2d.ap()[e * 128 : (e + 1) * 128, :])
            s["b2bc"] = b2bc
            v2n = pv.tile([128, NB * D], f16, tag="v2n", bufs=2, name=f"v2n_{e}")
            nc.sync.dma_start(out=v2n, in_=v2na[e * 128 : (e + 1) * 128, :])
            v1n = pv.tile([128, NB * D], bf16, tag="v1n", bufs=2, name=f"v1n_{e}")
            nc.sync.dma_start(out=v1n, in_=v1na[e * 128 : (e + 1) * 128, :])
            s["v2n"] = v2n
            s["v1n"] = v1n

        def stage_sim(e):
            s = st[e]
            m1nt = pst.tile([128, NB], f32, tag="m1nt", bufs=2, name=f"m1nt_{e}")
            s1t = pst.tile([128, NB], f32, tag="s1t", bufs=2, name=f"s1t_{e}")
            s["mk"], s["e1"], s["e2"] = [], [], []
            for ib in range(NB):
                ps = pps.tile([128, L], f32, tag="sim", bufs=2)
                for c in range(ND):
                    h, cc = divmod(c, NDH)
                    nc.tensor.matmul(
                        ps,
                        s["v1T"][h][:, cc * L + ib * 128 : cc * L + (ib + 1) * 128],
                        s["v2T"][h][:, cc * L : (cc + 1) * L],
                        start=(c == 0),
                        stop=(c == ND - 1),
                    )
                mk = pe_.tile([128, L], f32, tag="mk", bufs=2 * NB, name=f"mk_{e}_{ib}")
                nc.vector.tensor_add(mk, ps, s["b2bc"])
                # m1n = -rowmax(mk): the e1 exp bias, negated in one op
                nc.vector.tensor_reduce(m1nt[:, ib : ib + 1], mk, axis=AXX, op=MAX,
                                        negate=True)
                e1 = pe_.tile([128, L], f16, tag="e1", bufs=2 * NB, name=f"e1_{e}_{ib}")
                nc.scalar.activation(out=e1, in_=mk, func=EXP,
                                     bias=m1nt[:, ib : ib + 1], scale=1.0,
                                     accum_out=s1t[:, ib : ib + 1])
                s["mk"].append(mk)
                s["e1"].append(e1)
            # gm = global max = -min(m1n); all-reduce across partitions
            gmx = pst.tile([128, 1], f32, tag="gmx", bufs=2, name=f"gmx_{e}")
            nc.vector.tensor_reduce(gmx, m1nt, axis=AXX, op=MIN, negate=True)
            gmr = pst.tile([128, 1], f32, tag="gmr", bufs=2, name=f"gmr_{e}")
            nc.gpsimd.partition_all_reduce(gmr, gmx, 128, bass_isa.ReduceOp.max)
            # bias = 60 - gm keeps e2 numerators in normal fp32/bf16 range
            gmn = pst.tile([128, 1], f32, tag="gmn", bufs=2, name=f"gmn_{e}")
            nc.vector.tensor_scalar(gmn, gmr, -1.0, 60.0, op0=mybir.AluOpType.mult,
                                    op1=ADD)
            comb2 = pst.tile([128, NB], f32, tag="comb2", bufs=2, name=f"comb2_{e}")
            nc.vector.tensor_scalar_add(comb2, bcs[:, e * NB : e * NB + NB], gmn)
            r1t = pst.tile([128, NB], f32, tag="r1t", bufs=2, name=f"r1t_{e}")
            nc.vector.reciprocal(out=r1t, in_=s1t)
            sc1t = pst.tile([128, NB], f32, tag="sc1t", bufs=2, name=f"sc1t_{e}")
            nc.vector.tensor_mul(sc1t, r1t, cms[:, e * NB : e * NB + NB])
            s["sc1t"] = sc1t
            # e2 = exp(mk + b1col - gm + 60); b2row term cancels per-column
            for ib in range(NB):
                e2 = pe_.tile([128, L], bf16, tag="e2", bufs=2 * NB, name=f"e2_{e}_{ib}")
                nc.scalar.activation(out=e2, in_=s["mk"][ib], func=EXP,
                                     bias=comb2[:, ib : ib + 1], scale=1.0)
                s["e2"].append(e2)

        def stage_fin(e):
            s = st[e]
            # ---- transpose e1 into [j,i] lhsT layout (fp16 PE transposes)
            e1ji = pe_.tile([128, NB * L], f16, tag="e1ji", bufs=2, name=f"e1ji_{e}")
            for jb in range(NB):
                ps = pps.tile([128, L], f16, tag="pte", bufs=2, name=f"pt1_{e}_{jb}")
                for ib in range(NB):
                    nc.tensor.transpose(
                        ps[:, ib * 128 : (ib + 1) * 128],
                        s["e1"][ib][:, jb * 128 : (jb + 1) * 128],
                        ident,
                    )
                nc.vector.tensor_copy(e1ji[:, jb * L : (jb + 1) * L], ps)
            # ---- out1[i,d] = sum_j e1[j,i] v2[j,d] / s1, masked rows zeroed
            av1 = pav.tile([128, NB * D], bf16, tag="av1", bufs=2, name=f"av1_{e}")
            for ib in range(NB):
                for dc in range(NDC):
                    ps = pps.tile([128, 512], f32, tag="att", bufs=2)
                    for jb in range(NB):
                        nc.tensor.matmul(
                            ps,
                            e1ji[:, jb * L + ib * 128 : jb * L + (ib + 1) * 128],
                            s["v2n"][:, jb * D + dc * 512 : jb * D + (dc + 1) * 512],
                            start=(jb == 0),
                            stop=(jb == NB - 1),
                        )
                    dst = av1[:, ib * D + dc * 512 : ib * D + (dc + 1) * 512]
                    if dc == 0:
                        nc.scalar.activation(out=dst, in_=ps, func=COPY,
                                             scale=s["sc1t"][:, ib : ib + 1])
                    else:
                        nc.vector.tensor_scalar_mul(dst, ps, s["sc1t"][:, ib : ib + 1])
            nc.gpsimd.dma_start(out=o1a[e * 128 : (e + 1) * 128, :], in_=av1)
            # ---- s2 column sums via ones-matmuls on e2 tiles
            pss = pps.tile([128, 2 * NB], f32, tag="pss", bufs=2, name=f"pss_{e}")
            for jb in range(NB):
                for ib in range(NB):
                    nc.tensor.matmul(pss[:, 2 * jb : 2 * jb + 2],
                                     s["e2"][ib][:, jb * 128 : (jb + 1) * 128], onesr,
                                     start=(ib == 0), stop=(ib == NB - 1))
            s2t = pst.tile([128, NB], f32, tag="s2t", bufs=2, name=f"s2t_{e}")
            nc.vector.tensor_scalar_add(s2t, pss[:, 0 : 2 * NB : 2], 1.0e-36)
            r2t = pst.tile([128, NB], f32, tag="r2t", bufs=2, name=f"r2t_{e}")
            nc.vector.reciprocal(out=r2t, in_=s2t)
            sc2t = pst.tile([128, NB], f32, tag="sc2t", bufs=2, name=f"sc2t_{e}")
            nc.vector.tensor_mul(sc2t, r2t, cms[:, EPC * NB + e * NB : EPC * NB + e * NB + NB])
            # ---- out2[j,d] = sum_i e2[i,j] v1[i,d] / s2, masked rows zeroed
            av2 = pav.tile([128, NB * D], bf16, tag="av2", bufs=2, name=f"av2_{e}")
            for jb in range(NB):
                for dc in range(NDC):
                    ps = pps.tile([128, 512], f32, tag="att", bufs=2)
                    for ib in range(NB):
                        nc.tensor.matmul(
                            ps,
                            s["e2"][ib][:, jb * 128 : (jb + 1) * 128],
                            s["v1n"][:, ib * D + dc * 512 : ib * D + (dc + 1) * 512],
                            start=(ib == 0),
                            stop=(ib == NB - 1),
                        )
                    dst = av2[:, jb * D + dc * 512 : jb * D + (dc + 1) * 512]
                    if dc == 0:
                        nc.vector.tensor_scalar_mul(dst, ps, sc2t[:, jb : jb + 1])
                    else:
                        nc.scalar.activation(out=dst, in_=ps, func=COPY,
                                             scale=sc2t[:, jb : jb + 1])
            nc.gpsimd.dma_start(out=o2a[e * 128 : (e + 1) * 128, :], in_=av2)
            st[e] = {}

        stage_load(0)
        stage_sim(0)
        for e in range(1, EPC):
            stage_load(e)
            stage_sim(e)
            stage_fin(e - 1)
        stage_fin(EPC - 1)

    nc.compile()
    return nc


def get_nc():
    if "nc" not in _CACHE:
        _CACHE["nc"] = _build_nc()
    return _CACHE["nc"]


def _host_prep(v1, v2, v1_mask, v2_mask):
    """Build per-core input maps (packed per-example layouts) from full inputs."""
    import ml_dtypes

    bf16 = ml_dtypes.bfloat16
    v1 = np.asarray(v1, dtype=np.float32)
    v2 = np.asarray(v2, dtype=np.float32)
    v1_mask = np.asarray(v1_mask).astype(bool)
    v2_mask = np.asarray(v2_mask).astype(bool)

    def pack_t(x):  # [EPC, L, D] -> [EPC*128, ND*L]; row p = concat chunk rows
        return np.ascontiguousarray(
            x.transpose(0, 2, 1).reshape(EPC, ND, 128, L).transpose(0, 2, 1, 3)
            .reshape(EPC * 128, ND * L))

    def pack_n(x):  # [EPC, L, D] -> [EPC*128, NB*D]; row p = concat block rows
        return np.ascontiguousarray(
            x.reshape(EPC, NB, 128, D).transpose(0, 2, 1, 3).reshape(EPC * 128, NB * D))

    in_maps = []
    for k in range(NCORES):
        sl = slice(EPC * k, EPC * (k + 1))
        v1c, v2c = v1[sl], v2[sl]
        m1 = v1_mask[sl]
        m2 = v2_mask[sl]
        b1 = np.where(m1, np.float32(-1e30), np.float32(0.0)).astype(np.float32)
        b2 = np.where(m2, np.float32(-1e30), np.float32(0.0)).astype(np.float32)
        bcol = np.ascontiguousarray(b1.reshape(EPC, NB, 128).transpose(2, 0, 1).reshape(128, EPC * NB))
        b2rep = np.repeat(b2[:, None, :], 128, axis=1).reshape(EPC * 128, L)
        k1 = (~m1).astype(np.float32).reshape(EPC, NB, 128).transpose(2, 0, 1).reshape(128, EPC * NB)
        k2 = (~m2).astype(np.float32).reshape(EPC, NB, 128).transpose(2, 0, 1).reshape(128, EPC * NB)
        in_maps.append(
            {
                "v1t": pack_t(v1c).astype(np.float16),
                "v2t": pack_t(v2c).astype(np.float16),
                "v2n": pack_n(v2c).astype(np.float16),
                "v1n": pack_n(v1c).astype(bf16),
                "b2r": np.ascontiguousarray(b2rep),
                "bcol": bcol,
                "ones2": np.ones((128, 2), bf16),
                "cm": np.ascontiguousarray(np.concatenate([k1, k2], axis=1)),
                "idh": np.eye(128, dtype=np.float16),
            }
        )
    return in_maps


def kernel(v1, v2, v1_mask, v2_mask):
    global LAST_RESULTS
    from concourse.bass_utils import run_bass_kernel_spmd

    nc = get_nc()
    in_maps = _host_prep(v1, v2, v1_mask, v2_mask)
    res = run_bass_kernel_spmd(nc, in_maps, list(range(NCORES)))
    LAST_RESULTS = res

    def unpack(name):
        parts = []
        for k in range(NCORES):
            arr = res.results[k][name].astype(np.float32)
            parts.append(arr.reshape(EPC, 128, NB, D).transpose(0, 2, 1, 3).reshape(EPC, L, D))
        return np.concatenate(parts, axis=0)

    return unpack("o1"), unpack("o2")


# revision 21
# speedup vs baseline: 1.4993x; 1.0425x over previous
"""Bidirectional attention kernel for Trainium2 (8 NeuronCores, data-parallel over batch).

Math per example (B=32, L1=L2=512, D=1024):
    sim = v1 @ v2^T                                  [512, 512]
    attn1 = softmax_j(sim + v2maskbias)              (mask v2 cols)
    attn2 = softmax_i(sim + v1maskbias)              (mask v1 rows)
    out1  = (attn1 @ v2) zeroed at v1-masked rows    [512, 1024]
    out2  = (attn2^T @ v1) zeroed at v2-masked rows  [512, 1024]

Device strategy (4 examples per core), all-16-bit datapath:
  - sim matmul operands in fp16 (host pre-transposed); PSUM fp32. fp16
    logits keep softmax ties stable (bf16 does not: 9e-2 rel err).
  - e1 numerators fp16 (range (0,1]); e2 numerators bf16 (range up to
    e^60 from the global-max-bound trick, needs fp32 exponent range).
  - attend rhs: v2 natural fp16 (out1), v1 natural bf16 (out2). Outputs
    bf16, unpacked/upcast on host.
  - Host packs each per-example tensor into a single [128, x] row-major
    region so every load/store is ONE large DMA (DMA issue instructions
    cost ~650ns each on the issuing engine and serialize).
  - e1 -> e1ji transpose on the PE (fp16 transposes run 1 cyc/row, half
    the fp32 cost); PSUM->SBUF copies balanced across ACT and DVE
    (gpsimd/Pool cannot touch PSUM); store issues on gpsimd.
  - Row softmax stats ride the EXP activation accumulator; 1/sum and
    mask-zeroing fold into PSUM->SBUF output copies (per-partition
    scale) which are spread across ACT/DVE/Pool to keep all three off
    the critical path. Column sums for attn2 via tiny ones-matmuls.
  - 1-example software-pipeline skew: sim+softmax of example e issue
    before the attend matmuls of example e-1 so the PE never waits on
    the softmax stats chain and stays at full p-state.
"""

import numpy as np

B, L, D = 32, 512, 1024
NCORES = 8
EPC = B // NCORES  # examples per core
NB = L // 128      # 128-row blocks per L
ND = D // 128      # 128-row chunks per D (transposed layouts)
NDC = D // 512     # 512-col halves per D
NDH = ND // 2      # chunks per load half

_CACHE = {}
LAST_RESULTS = None


def _build_nc():
    from contextlib import ExitStack
    import concourse.bacc as bacc
    import concourse.tile as tile
    import concourse.mybir as mybir
    import concourse.bass_isa as bass_isa

    f32 = mybir.dt.float32
    f16 = mybir.dt.float16
    bf16 = mybir.dt.bfloat16
    EXP = mybir.ActivationFunctionType.Exp
    COPY = mybir.ActivationFunctionType.Copy
    ADD = mybir.AluOpType.add
    MIN = mybir.AluOpType.min
    MAX = mybir.AluOpType.max
    AXX = mybir.AxisListType.X

    nc = bacc.Bacc("TRN2", target_bir_lowering=False, debug=False, num_devices=NCORES)
    # packed layouts: one [128, x] row-major region per example per tensor
    v1td = nc.dram_tensor("v1t", [EPC * 128, ND * L], f16, kind="ExternalInput")
    v2td = nc.dram_tensor("v2t", [EPC * 128, ND * L], f16, kind="ExternalInput")
    v2nd = nc.dram_tensor("v2n", [EPC * 128, NB * D], f16, kind="ExternalInput")
    v1nd = nc.dram_tensor("v1n", [EPC * 128, NB * D], bf16, kind="ExternalInput")
    b2d = nc.dram_tensor("b2r", [EPC * 128, L], f32, kind="ExternalInput")
    cmd = nc.dram_tensor("cm", [128, 2 * EPC * NB], f32, kind="ExternalInput")
    bcd = nc.dram_tensor("bcol", [128, EPC * NB], f32, kind="ExternalInput")
    idd = nc.dram_tensor("idh", [128, 128], f16, kind="ExternalInput")
    ond = nc.dram_tensor("ones2", [128, 2], bf16, kind="ExternalInput")
    o1d = nc.dram_tensor("o1", [EPC * 128, NB * D], bf16, kind="ExternalOutput")
    o2d = nc.dram_tensor("o2", [EPC * 128, NB * D], bf16, kind="ExternalOutput")
    v1ta, v2ta, v2na, v1na = v1td.ap(), v2td.ap(), v2nd.ap(), v1nd.ap()
    o1a, o2a = o1d.ap(), o2d.ap()

    with ExitStack() as ctx:
        tc = ctx.enter_context(tile.TileContext(nc))
        const = ctx.enter_context(tc.tile_pool(name="const", bufs=1))
        pv = ctx.enter_context(tc.tile_pool(name="pv", bufs=1))
        pvt = ctx.enter_context(tc.tile_pool(name="pvt", bufs=1))
        pe_ = ctx.enter_context(tc.tile_pool(name="pe", bufs=1))
        pst = ctx.enter_context(tc.tile_pool(name="pst", bufs=1))
        pbb = ctx.enter_context(tc.tile_pool(name="pbb", bufs=1))
        pav = ctx.enter_context(tc.tile_pool(name="pav", bufs=1))
        pps = ctx.enter_context(tc.tile_pool(name="pps", bufs=1, space="PSUM"))

        ident = const.tile([128, 128], f16)
        nc.sync.dma_start(out=ident, in_=idd.ap())
        cms = const.tile([128, 2 * EPC * NB], f32)
        nc.sync.dma_start(out=cms, in_=cmd.ap())
        bcs = const.tile([128, EPC * NB], f32)
        nc.sync.dma_start(out=bcs, in_=bcd.ap())
        onesr = const.tile([128, 2], bf16)
        nc.sync.dma_start(out=onesr, in_=ond.ap())

        st = [dict() for _ in range(EPC)]

        def stage_load(e):
            s = st[e]
            # one [128, 8*512] tile per transposed tensor, filled by 4
            # quarter-DMAs (2 chunks each) so the first sim matmuls start
            # ~1.5us after launch instead of waiting for the full 1MB
            v1T = pvt.tile([128, ND * L], f16, tag="v1T", bufs=2, name=f"v1T_{e}")
            v2T = pvt.tile([128, ND * L], f16, tag="v2T", bufs=2, name=f"v2T_{e}")
            QW = 2 * L  # quarter width: 2 chunks
            for q in range(4):
                nc.sync.dma_start(out=v1T[:, q * QW : (q + 1) * QW],
                                  in_=v1ta[e * 128 : (e + 1) * 128, q * QW : (q + 1) * QW])
                nc.sync.dma_start(out=v2T[:, q * QW : (q + 1) * QW],
                                  in_=v2ta[e * 128 : (e + 1) * 128, q * QW : (q + 1) * QW])
            s["v1T"] = v1T
            s["v2T"] = v2T
            b2bc = pbb.tile([128, L], f32, tag="b2", bufs=2, name=f"b2bc_{e}")
            nc.sync.dma_start(out=b2bc, in_=b2d.ap()[e * 128 : (e + 1) * 128, :])
            s["b2bc"] = b2bc
            v2n = pv.tile([128, NB * D], f16, tag="v2n", bufs=2, name=f"v2n_{e}")
            nc.sync.dma_start(out=v2n, in_=v2na[e * 128 : (e + 1) * 128, :])
            v1n = pv.tile([128, NB * D], bf16, tag="v1n", bufs=2, name=f"v1n_{e}")
            nc.sync.dma_start(out=v1n, in_=v1na[e * 128 : (e + 1) * 128, :])
            s["v2n"] = v2n
            s["v1n"] = v1n

        def stage_sim(e):
            s = st[e]
            m1nt = pst.tile([128, NB], f32, tag="m1nt", bufs=2, name=f"m1nt_{e}")
            s1t = pst.tile([128, NB], f32, tag="s1t", bufs=2, name=f"s1t_{e}")
            s["mk"], s["e1"], s["e2"] = [], [], []
            for ib in range(NB):
                ps = pps.tile([128, L], f32, tag="sim", bufs=2)
                for c in range(ND):
                    nc.tensor.matmul(
                        ps,
                        s["v1T"][:, c * L + ib * 128 : c * L + (ib + 1) * 128],
                        s["v2T"][:, c * L : (c + 1) * L],
                        start=(c == 0),
                        stop=(c == ND - 1),
                    )
                mk = pe_.tile([128, L], f32, tag="mk", bufs=2 * NB, name=f"mk_{e}_{ib}")
                nc.vector.tensor_add(mk, ps, s["b2bc"])
                # m1n = -rowmax(mk): the e1 exp bias, negated in one op
                nc.vector.tensor_reduce(m1nt[:, ib : ib + 1], mk, axis=AXX, op=MAX,
                                        negate=True)
                e1 = pe_.tile([128, L], f16, tag="e1", bufs=2 * NB, name=f"e1_{e}_{ib}")
                nc.scalar.activation(out=e1, in_=mk, func=EXP,
                                     bias=m1nt[:, ib : ib + 1], scale=1.0,
                                     accum_out=s1t[:, ib : ib + 1])
                s["mk"].append(mk)
                s["e1"].append(e1)
            # gm = global max = -min(m1n); all-reduce across partitions
            gmx = pst.tile([128, 1], f32, tag="gmx", bufs=2, name=f"gmx_{e}")
            nc.vector.tensor_reduce(gmx, m1nt, axis=AXX, op=MIN, negate=True)
            gmr = pst.tile([128, 1], f32, tag="gmr", bufs=2, name=f"gmr_{e}")
            nc.gpsimd.partition_all_reduce(gmr, gmx, 128, bass_isa.ReduceOp.max)
            # bias = 60 - gm keeps e2 numerators in normal fp32/bf16 range
            gmn = pst.tile([128, 1], f32, tag="gmn", bufs=2, name=f"gmn_{e}")
            nc.vector.tensor_scalar(gmn, gmr, -1.0, 60.0, op0=mybir.AluOpType.mult,
                                    op1=ADD)
            comb2 = pst.tile([128, NB], f32, tag="comb2", bufs=2, name=f"comb2_{e}")
            nc.vector.tensor_scalar_add(comb2, bcs[:, e * NB : e * NB + NB], gmn)
            r1t = pst.tile([128, NB], f32, tag="r1t", bufs=2, name=f"r1t_{e}")
            nc.vector.reciprocal(out=r1t, in_=s1t)
            sc1t = pst.tile([128, NB], f32, tag="sc1t", bufs=2, name=f"sc1t_{e}")
            nc.vector.tensor_mul(sc1t, r1t, cms[:, e * NB : e * NB + NB])
            s["sc1t"] = sc1t
            # e2 = exp(mk + b1col - gm + 60); b2row term cancels per-column
            for ib in range(NB):
                e2 = pe_.tile([128, L], bf16, tag="e2", bufs=2 * NB, name=f"e2_{e}_{ib}")
                nc.scalar.activation(out=e2, in_=s["mk"][ib], func=EXP,
                                     bias=comb2[:, ib : ib + 1], scale=1.0)
                s["e2"].append(e2)

        def stage_finA(e):
            # transpose e1 into [j,i] lhsT layout (fp16 PE transposes).
            # Emitted BEFORE sim(e+1) so the DVE copies don't queue behind
            # the next example's softmax work (e1 deps are long satisfied).
            s = st[e]
            e1ji = pe_.tile([128, NB * L], f16, tag="e1ji", bufs=2, name=f"e1ji_{e}")
            for jb in range(NB):
                ps = pps.tile([128, L], f16, tag="pte", bufs=2, name=f"pt1_{e}_{jb}")
                for ib in range(NB):
                    nc.tensor.transpose(
                        ps[:, ib * 128 : (ib + 1) * 128],
                        s["e1"][ib][:, jb * 128 : (jb + 1) * 128],
                        ident,
                    )
                nc.vector.tensor_copy(e1ji[:, jb * L : (jb + 1) * L], ps)
            s["e1ji"] = e1ji

        def stage_finB(e):
            s = st[e]
            e1ji = s["e1ji"]
            # ---- out1[i,d] = sum_j e1[j,i] v2[j,d] / s1, masked rows zeroed
            for ib in range(NB):
                av = pav.tile([128, D], bf16, tag="av1", bufs=3)
                for dc in range(NDC):
                    ps = pps.tile([128, 512], f32, tag="att", bufs=3)
                    for jb in range(NB):
                        nc.tensor.matmul(
                            ps,
                            e1ji[:, jb * L + ib * 128 : jb * L + (ib + 1) * 128],
                            s["v2n"][:, jb * D + dc * 512 : jb * D + (dc + 1) * 512],
                            start=(jb == 0),
                            stop=(jb == NB - 1),
                        )
                    dst = av[:, dc * 512 : (dc + 1) * 512]
                    if dc == 0:
                        nc.scalar.activation(out=dst, in_=ps, func=COPY,
                                             scale=s["sc1t"][:, ib : ib + 1])
                    else:
                        nc.vector.tensor_scalar_mul(dst, ps, s["sc1t"][:, ib : ib + 1])
                nc.gpsimd.dma_start(
                    out=o1a[e * 128 : (e + 1) * 128, ib * D : (ib + 1) * D], in_=av)
            # ---- s2 column sums via ones-matmuls on e2 tiles
            pss = pps.tile([128, 2 * NB], f32, tag="pss", bufs=1, name=f"pss_{e}")
            for jb in range(NB):
                for ib in range(NB):
                    nc.tensor.matmul(pss[:, 2 * jb : 2 * jb + 2],
                                     s["e2"][ib][:, jb * 128 : (jb + 1) * 128], onesr,
                                     start=(ib == 0), stop=(ib == NB - 1))
            s2t = pst.tile([128, NB], f32, tag="s2t", bufs=2, name=f"s2t_{e}")
            nc.vector.tensor_scalar_add(s2t, pss[:, 0 : 2 * NB : 2], 1.0e-36)
            r2t = pst.tile([128, NB], f32, tag="r2t", bufs=2, name=f"r2t_{e}")
            nc.vector.reciprocal(out=r2t, in_=s2t)
            sc2t = pst.tile([128, NB], f32, tag="sc2t", bufs=2, name=f"sc2t_{e}")
            nc.vector.tensor_mul(sc2t, r2t, cms[:, EPC * NB + e * NB : EPC * NB + e * NB + NB])
            # ---- out2[j,d] = sum_i e2[i,j] v1[i,d] / s2, masked rows zeroed
            for jb in range(NB):
                av = pav.tile([128, D], bf16, tag="av2", bufs=3)
                for dc in range(NDC):
                    ps = pps.tile([128, 512], f32, tag="att", bufs=3)
                    for ib in range(NB):
                        nc.tensor.matmul(
                            ps,
                            s["e2"][ib][:, jb * 128 : (jb + 1) * 128],
                            s["v1n"][:, ib * D + dc * 512 : ib * D + (dc + 1) * 512],
                            start=(ib == 0),
                            stop=(ib == NB - 1),
                        )
                    dst = av[:, dc * 512 : (dc + 1) * 512]
                    if dc == 0:
                        nc.vector.tensor_scalar_mul(dst, ps, sc2t[:, jb : jb + 1])
                    else:
                        nc.scalar.activation(out=dst, in_=ps, func=COPY,
                                             scale=sc2t[:, jb : jb + 1])
                nc.gpsimd.dma_start(
                    out=o2a[e * 128 : (e + 1) * 128, jb * D : (jb + 1) * D], in_=av)
            st[e] = {}

        stage_load(0)
        stage_sim(0)
        for e in range(1, EPC):
            stage_load(e)
            stage_finA(e - 1)
            stage_sim(e)
            stage_finB(e - 1)
        stage_finA(EPC - 1)
        stage_finB(EPC - 1)

    nc.compile()
    return nc


def get_nc():
    if "nc" not in _CACHE:
        _CACHE["nc"] = _build_nc()
    return _CACHE["nc"]


def _host_prep(v1, v2, v1_mask, v2_mask):
    """Build per-core input maps (packed per-example layouts) from full inputs."""
    import ml_dtypes

    bf16 = ml_dtypes.bfloat16
    v1 = np.asarray(v1, dtype=np.float32)
    v2 = np.asarray(v2, dtype=np.float32)
    v1_mask = np.asarray(v1_mask).astype(bool)
    v2_mask = np.asarray(v2_mask).astype(bool)

    def pack_t(x):  # [EPC, L, D] -> [EPC*128, ND*L]; row p = concat chunk rows
        return np.ascontiguousarray(
            x.transpose(0, 2, 1).reshape(EPC, ND, 128, L).transpose(0, 2, 1, 3)
            .reshape(EPC * 128, ND * L))

    def pack_n(x):  # [EPC, L, D] -> [EPC*128, NB*D]; row p = concat block rows
        return np.ascontiguousarray(
            x.reshape(EPC, NB, 128, D).transpose(0, 2, 1, 3).reshape(EPC * 128, NB * D))

    in_maps = []
    for k in range(NCORES):
        sl = slice(EPC * k, EPC * (k + 1))
        v1c, v2c = v1[sl], v2[sl]
        m1 = v1_mask[sl]
        m2 = v2_mask[sl]
        b1 = np.where(m1, np.float32(-1e30), np.float32(0.0)).astype(np.float32)
        b2 = np.where(m2, np.float32(-1e30), np.float32(0.0)).astype(np.float32)
        bcol = np.ascontiguousarray(b1.reshape(EPC, NB, 128).transpose(2, 0, 1).reshape(128, EPC * NB))
        b2rep = np.repeat(b2[:, None, :], 128, axis=1).reshape(EPC * 128, L)
        k1 = (~m1).astype(np.float32).reshape(EPC, NB, 128).transpose(2, 0, 1).reshape(128, EPC * NB)
        k2 = (~m2).astype(np.float32).reshape(EPC, NB, 128).transpose(2, 0, 1).reshape(128, EPC * NB)
        in_maps.append(
            {
                "v1t": pack_t(v1c).astype(np.float16),
                "v2t": pack_t(v2c).astype(np.float16),
                "v2n": pack_n(v2c).astype(np.float16),
                "v1n": pack_n(v1c).astype(bf16),
                "b2r": np.ascontiguousarray(b2rep),
                "bcol": bcol,
                "ones2": np.ones((128, 2), bf16),
                "cm": np.ascontiguousarray(np.concatenate([k1, k2], axis=1)),
                "idh": np.eye(128, dtype=np.float16),
            }
        )
    return in_maps


def kernel(v1, v2, v1_mask, v2_mask):
    global LAST_RESULTS
    from concourse.bass_utils import run_bass_kernel_spmd

    nc = get_nc()
    in_maps = _host_prep(v1, v2, v1_mask, v2_mask)
    res = run_bass_kernel_spmd(nc, in_maps, list(range(NCORES)))
    LAST_RESULTS = res

    def unpack(name):
        parts = []
        for k in range(NCORES):
            arr = res.results[k][name].astype(np.float32)
            parts.append(arr.reshape(EPC, 128, NB, D).transpose(0, 2, 1, 3).reshape(EPC, L, D))
        return np.concatenate(parts, axis=0)

    return unpack("o1"), unpack("o2")


# revision 24
# speedup vs baseline: 1.5195x; 1.0135x over previous
"""Bidirectional attention kernel for Trainium2 (8 NeuronCores, data-parallel over batch).

Math per example (B=32, L1=L2=512, D=1024):
    sim = v1 @ v2^T                                  [512, 512]
    attn1 = softmax_j(sim + v2maskbias)              (mask v2 cols)
    attn2 = softmax_i(sim + v1maskbias)              (mask v1 rows)
    out1  = (attn1 @ v2) zeroed at v1-masked rows    [512, 1024]
    out2  = (attn2^T @ v1) zeroed at v2-masked rows  [512, 1024]

Device strategy (4 examples per core), all-16-bit datapath:
  - sim matmul operands in fp16 (host pre-transposed); PSUM fp32. fp16
    logits keep softmax ties stable (bf16 does not: 9e-2 rel err).
  - e1 numerators fp16 (range (0,1]); e2 numerators bf16 (range up to
    e^60 from the global-max-bound trick, needs fp32 exponent range).
  - attend rhs: v2 natural fp16 (out1), v1 natural bf16 (out2). Outputs
    bf16, unpacked/upcast on host.
  - Host packs each per-example tensor into a single [128, x] row-major
    region so every load/store is ONE large DMA (DMA issue instructions
    cost ~650ns each on the issuing engine and serialize).
  - e1 -> e1ji transpose on the PE (fp16 transposes run 1 cyc/row, half
    the fp32 cost); PSUM->SBUF copies balanced across ACT and DVE
    (gpsimd/Pool cannot touch PSUM); store issues on gpsimd.
  - Row softmax stats ride the EXP activation accumulator; 1/sum and
    mask-zeroing fold into PSUM->SBUF output copies (per-partition
    scale) which are spread across ACT/DVE/Pool to keep all three off
    the critical path. Column sums for attn2 via tiny ones-matmuls.
  - 1-example software-pipeline skew: sim+softmax of example e issue
    before the attend matmuls of example e-1 so the PE never waits on
    the softmax stats chain and stays at full p-state.
"""

import numpy as np

B, L, D = 32, 512, 1024
NCORES = 8
EPC = B // NCORES  # examples per core
NB = L // 128      # 128-row blocks per L
ND = D // 128      # 128-row chunks per D (transposed layouts)
NDC = D // 512     # 512-col halves per D
NDH = ND // 2      # chunks per load half

_CACHE = {}
LAST_RESULTS = None


def _build_nc():
    from contextlib import ExitStack
    import concourse.bacc as bacc
    import concourse.tile as tile
    import concourse.mybir as mybir
    import concourse.bass_isa as bass_isa

    f32 = mybir.dt.float32
    f16 = mybir.dt.float16
    bf16 = mybir.dt.bfloat16
    EXP = mybir.ActivationFunctionType.Exp
    COPY = mybir.ActivationFunctionType.Copy
    ADD = mybir.AluOpType.add
    MIN = mybir.AluOpType.min
    MAX = mybir.AluOpType.max
    AXX = mybir.AxisListType.X

    nc = bacc.Bacc("TRN2", target_bir_lowering=False, debug=False, num_devices=NCORES)
    # packed layouts: one [128, x] row-major region per example per tensor
    v1td = nc.dram_tensor("v1t", [EPC * 128, ND * L], f16, kind="ExternalInput")
    v2td = nc.dram_tensor("v2t", [EPC * 128, ND * L], f16, kind="ExternalInput")
    v2nd = nc.dram_tensor("v2n", [EPC * 128, NB * D], f16, kind="ExternalInput")
    v1nd = nc.dram_tensor("v1n", [EPC * 128, NB * D], bf16, kind="ExternalInput")
    b2d = nc.dram_tensor("b2r", [EPC * 128, L], f32, kind="ExternalInput")
    cmd = nc.dram_tensor("cm", [128, 2 * EPC * NB], f32, kind="ExternalInput")
    bcd = nc.dram_tensor("bcol", [128, EPC * NB], f32, kind="ExternalInput")
    idd = nc.dram_tensor("idh", [128, 128], f16, kind="ExternalInput")
    ond = nc.dram_tensor("ones2", [128, 2], bf16, kind="ExternalInput")
    o1d = nc.dram_tensor("o1", [EPC * 128, NB * D], bf16, kind="ExternalOutput")
    o2d = nc.dram_tensor("o2", [EPC * 128, NB * D], bf16, kind="ExternalOutput")
    v1ta, v2ta, v2na, v1na = v1td.ap(), v2td.ap(), v2nd.ap(), v1nd.ap()
    o1a, o2a = o1d.ap(), o2d.ap()

    with ExitStack() as ctx:
        tc = ctx.enter_context(tile.TileContext(nc))
        const = ctx.enter_context(tc.tile_pool(name="const", bufs=1))
        pv = ctx.enter_context(tc.tile_pool(name="pv", bufs=1))
        pvt = ctx.enter_context(tc.tile_pool(name="pvt", bufs=1))
        pe_ = ctx.enter_context(tc.tile_pool(name="pe", bufs=1))
        pst = ctx.enter_context(tc.tile_pool(name="pst", bufs=1))
        pbb = ctx.enter_context(tc.tile_pool(name="pbb", bufs=1))
        pav = ctx.enter_context(tc.tile_pool(name="pav", bufs=1))
        pps = ctx.enter_context(tc.tile_pool(name="pps", bufs=1, space="PSUM"))

        ident = const.tile([128, 128], f16)
        nc.sync.dma_start(out=ident, in_=idd.ap())
        cms = const.tile([128, 2 * EPC * NB], f32)
        nc.sync.dma_start(out=cms, in_=cmd.ap())
        bcs = const.tile([128, EPC * NB], f32)
        nc.sync.dma_start(out=bcs, in_=bcd.ap())
        onesr = const.tile([128, 2], bf16)
        nc.sync.dma_start(out=onesr, in_=ond.ap())

        st = [dict() for _ in range(EPC)]

        def stage_load(e):
            s = st[e]
            # one [128, 8*512] tile per transposed tensor, filled by 4
            # quarter-DMAs (2 chunks each) so the first sim matmuls start
            # ~1.5us after launch instead of waiting for the full 1MB
            # b2 bias first: the DVE mk-add needs it to free sim PSUM banks,
            # so it must not queue behind 2MB of sim operands at startup
            b2bc0 = pbb.tile([128, L], f32, tag="b2", bufs=2, name=f"b2bc_{e}")
            nc.sync.dma_start(out=b2bc0, in_=b2d.ap()[e * 128 : (e + 1) * 128, :])
            v1T = pvt.tile([128, ND * L], f16, tag="v1T", bufs=2, name=f"v1T_{e}")
            v2T = pvt.tile([128, ND * L], f16, tag="v2T", bufs=2, name=f"v2T_{e}")
            QW = 2 * L  # quarter width: 2 chunks
            for q in range(4):
                nc.sync.dma_start(out=v1T[:, q * QW : (q + 1) * QW],
                                  in_=v1ta[e * 128 : (e + 1) * 128, q * QW : (q + 1) * QW])
                nc.sync.dma_start(out=v2T[:, q * QW : (q + 1) * QW],
                                  in_=v2ta[e * 128 : (e + 1) * 128, q * QW : (q + 1) * QW])
            s["v1T"] = v1T
            s["v2T"] = v2T
            s["b2bc"] = b2bc0
            v2n = pv.tile([128, NB * D], f16, tag="v2n", bufs=2, name=f"v2n_{e}")
            nc.sync.dma_start(out=v2n, in_=v2na[e * 128 : (e + 1) * 128, :])
            v1n = pv.tile([128, NB * D], bf16, tag="v1n", bufs=2, name=f"v1n_{e}")
            nc.sync.dma_start(out=v1n, in_=v1na[e * 128 : (e + 1) * 128, :])
            s["v2n"] = v2n
            s["v1n"] = v1n

        def stage_sim(e):
            s = st[e]
            m1nt = pst.tile([128, NB], f32, tag="m1nt", bufs=2, name=f"m1nt_{e}")
            s1t = pst.tile([128, NB], f32, tag="s1t", bufs=2, name=f"s1t_{e}")
            s["mk"], s["e1"], s["e2"] = [], [], []
            for ib in range(NB):
                ps = pps.tile([128, L], f32, tag="sim", bufs=2)
                for c in range(ND):
                    nc.tensor.matmul(
                        ps,
                        s["v1T"][:, c * L + ib * 128 : c * L + (ib + 1) * 128],
                        s["v2T"][:, c * L : (c + 1) * L],
                        start=(c == 0),
                        stop=(c == ND - 1),
                    )
                mk = pe_.tile([128, L], f32, tag="mk", bufs=2 * NB, name=f"mk_{e}_{ib}")
                nc.vector.tensor_add(mk, ps, s["b2bc"])
                # m1n = -rowmax(mk): the e1 exp bias, negated in one op
                nc.vector.tensor_reduce(m1nt[:, ib : ib + 1], mk, axis=AXX, op=MAX,
                                        negate=True)
                e1 = pe_.tile([128, L], f16, tag="e1", bufs=2 * NB, name=f"e1_{e}_{ib}")
                nc.scalar.activation(out=e1, in_=mk, func=EXP,
                                     bias=m1nt[:, ib : ib + 1], scale=1.0,
                                     accum_out=s1t[:, ib : ib + 1])
                s["mk"].append(mk)
                s["e1"].append(e1)
            # gm = global max = -min(m1n); all-reduce across partitions
            gmx = pst.tile([128, 1], f32, tag="gmx", bufs=2, name=f"gmx_{e}")
            nc.vector.tensor_reduce(gmx, m1nt, axis=AXX, op=MIN, negate=True)
            gmr = pst.tile([128, 1], f32, tag="gmr", bufs=2, name=f"gmr_{e}")
            nc.gpsimd.partition_all_reduce(gmr, gmx, 128, bass_isa.ReduceOp.max)
            # bias = 60 - gm keeps e2 numerators in normal fp32/bf16 range
            gmn = pst.tile([128, 1], f32, tag="gmn", bufs=2, name=f"gmn_{e}")
            nc.vector.tensor_scalar(gmn, gmr, -1.0, 60.0, op0=mybir.AluOpType.mult,
                                    op1=ADD)
            comb2 = pst.tile([128, NB], f32, tag="comb2", bufs=2, name=f"comb2_{e}")
            nc.vector.tensor_scalar_add(comb2, bcs[:, e * NB : e * NB + NB], gmn)
            r1t = pst.tile([128, NB], f32, tag="r1t", bufs=2, name=f"r1t_{e}")
            nc.vector.reciprocal(out=r1t, in_=s1t)
            sc1t = pst.tile([128, NB], f32, tag="sc1t", bufs=2, name=f"sc1t_{e}")
            nc.vector.tensor_mul(sc1t, r1t, cms[:, e * NB : e * NB + NB])
            s["sc1t"] = sc1t
            # e2 = exp(mk + b1col - gm + 60); b2row term cancels per-column
            for ib in range(NB):
                e2 = pe_.tile([128, L], bf16, tag="e2", bufs=2 * NB, name=f"e2_{e}_{ib}")
                nc.scalar.activation(out=e2, in_=s["mk"][ib], func=EXP,
                                     bias=comb2[:, ib : ib + 1], scale=1.0)
                s["e2"].append(e2)

        def stage_finA(e):
            # transpose e1 into [j,i] lhsT layout (fp16 PE transposes).
            # Emitted BEFORE sim(e+1) so the DVE copies don't queue behind
            # the next example's softmax work (e1 deps are long satisfied).
            s = st[e]
            e1ji = pe_.tile([128, NB * L], f16, tag="e1ji", bufs=2, name=f"e1ji_{e}")
            for jb in range(NB):
                ps = pps.tile([128, L], f16, tag="pte", bufs=2, name=f"pt1_{e}_{jb}")
                for ib in range(NB):
                    nc.tensor.transpose(
                        ps[:, ib * 128 : (ib + 1) * 128],
                        s["e1"][ib][:, jb * 128 : (jb + 1) * 128],
                        ident,
                    )
                nc.vector.tensor_copy(e1ji[:, jb * L : (jb + 1) * L], ps)
            s["e1ji"] = e1ji

        def stage_finB(e):
            s = st[e]
            e1ji = s["e1ji"]
            # ---- out1[i,d] = sum_j e1[j,i] v2[j,d] / s1, masked rows zeroed
            for ib in range(NB):
                av = pav.tile([128, D], bf16, tag="av1", bufs=3)
                for dc in range(NDC):
                    ps = pps.tile([128, 512], f32, tag="att", bufs=3)
                    for jb in range(NB):
                        nc.tensor.matmul(
                            ps,
                            e1ji[:, jb * L + ib * 128 : jb * L + (ib + 1) * 128],
                            s["v2n"][:, jb * D + dc * 512 : jb * D + (dc + 1) * 512],
                            start=(jb == 0),
                            stop=(jb == NB - 1),
                        )
                    dst = av[:, dc * 512 : (dc + 1) * 512]
                    if dc == 0:
                        nc.scalar.activation(out=dst, in_=ps, func=COPY,
                                             scale=s["sc1t"][:, ib : ib + 1])
                    else:
                        nc.vector.tensor_scalar_mul(dst, ps, s["sc1t"][:, ib : ib + 1])
                nc.gpsimd.dma_start(
                    out=o1a[e * 128 : (e + 1) * 128, ib * D : (ib + 1) * D], in_=av)
            # ---- s2 column sums via ones-matmuls on e2 tiles
            pss = pps.tile([128, 2 * NB], f32, tag="pss", bufs=1, name=f"pss_{e}")
            for jb in range(NB):
                for ib in range(NB):
                    nc.tensor.matmul(pss[:, 2 * jb : 2 * jb + 2],
                                     s["e2"][ib][:, jb * 128 : (jb + 1) * 128], onesr,
                                     start=(ib == 0), stop=(ib == NB - 1))
            s2t = pst.tile([128, NB], f32, tag="s2t", bufs=2, name=f"s2t_{e}")
            nc.vector.tensor_scalar_add(s2t, pss[:, 0 : 2 * NB : 2], 1.0e-36)
            r2t = pst.tile([128, NB], f32, tag="r2t", bufs=2, name=f"r2t_{e}")
            nc.vector.reciprocal(out=r2t, in_=s2t)
            sc2t = pst.tile([128, NB], f32, tag="sc2t", bufs=2, name=f"sc2t_{e}")
            nc.vector.tensor_mul(sc2t, r2t, cms[:, EPC * NB + e * NB : EPC * NB + e * NB + NB])
            # ---- out2[j,d] = sum_i e2[i,j] v1[i,d] / s2, masked rows zeroed
            for jb in range(NB):
                av = pav.tile([128, D], bf16, tag="av2", bufs=3)
                for dc in range(NDC):
                    ps = pps.tile([128, 512], f32, tag="att", bufs=3)
                    for ib in range(NB):
                        nc.tensor.matmul(
                            ps,
                            s["e2"][ib][:, jb * 128 : (jb + 1) * 128],
                            s["v1n"][:, ib * D + dc * 512 : ib * D + (dc + 1) * 512],
                            start=(ib == 0),
                            stop=(ib == NB - 1),
                        )
                    dst = av[:, dc * 512 : (dc + 1) * 512]
                    if dc == 0:
                        nc.vector.tensor_scalar_mul(dst, ps, sc2t[:, jb : jb + 1])
                    else:
                        nc.scalar.activation(out=dst, in_=ps, func=COPY,
                                             scale=sc2t[:, jb : jb + 1])
                    if e == EPC - 1:
                        # drain the kernel tail: store each half as soon as
                        # its copy lands instead of waiting for the block
                        nc.gpsimd.dma_start(
                            out=o2a[e * 128 : (e + 1) * 128,
                                    jb * D + dc * 512 : jb * D + (dc + 1) * 512],
                            in_=dst)
                if e != EPC - 1:
                    nc.gpsimd.dma_start(
                        out=o2a[e * 128 : (e + 1) * 128, jb * D : (jb + 1) * D], in_=av)
            st[e] = {}

        stage_load(0)
        stage_sim(0)
        for e in range(1, EPC):
            stage_load(e)
            stage_finA(e - 1)
            stage_sim(e)
            stage_finB(e - 1)
        stage_finA(EPC - 1)
        stage_finB(EPC - 1)

    nc.compile()
    return nc


def get_nc():
    if "nc" not in _CACHE:
        _CACHE["nc"] = _build_nc()
    return _CACHE["nc"]


def _host_prep(v1, v2, v1_mask, v2_mask):
    """Build per-core input maps (packed per-example layouts) from full inputs."""
    import ml_dtypes

    bf16 = ml_dtypes.bfloat16
    v1 = np.asarray(v1, dtype=np.float32)
    v2 = np.asarray(v2, dtype=np.float32)
    v1_mask = np.asarray(v1_mask).astype(bool)
    v2_mask = np.asarray(v2_mask).astype(bool)

    def pack_t(x):  # [EPC, L, D] -> [EPC*128, ND*L]; row p = concat chunk rows
        return np.ascontiguousarray(
            x.transpose(0, 2, 1).reshape(EPC, ND, 128, L).transpose(0, 2, 1, 3)
            .reshape(EPC * 128, ND * L))

    def pack_n(x):  # [EPC, L, D] -> [EPC*128, NB*D]; row p = concat block rows
        return np.ascontiguousarray(
            x.reshape(EPC, NB, 128, D).transpose(0, 2, 1, 3).reshape(EPC * 128, NB * D))

    in_maps = []
    for k in range(NCORES):
        sl = slice(EPC * k, EPC * (k + 1))
        v1c, v2c = v1[sl], v2[sl]
        m1 = v1_mask[sl]
        m2 = v2_mask[sl]
        b1 = np.where(m1, np.float32(-1e30), np.float32(0.0)).astype(np.float32)
        b2 = np.where(m2, np.float32(-1e30), np.float32(0.0)).astype(np.float32)
        bcol = np.ascontiguousarray(b1.reshape(EPC, NB, 128).transpose(2, 0, 1).reshape(128, EPC * NB))
        b2rep = np.repeat(b2[:, None, :], 128, axis=1).reshape(EPC * 128, L)
        k1 = (~m1).astype(np.float32).reshape(EPC, NB, 128).transpose(2, 0, 1).reshape(128, EPC * NB)
        k2 = (~m2).astype(np.float32).reshape(EPC, NB, 128).transpose(2, 0, 1).reshape(128, EPC * NB)
        in_maps.append(
            {
                "v1t": pack_t(v1c).astype(np.float16),
                "v2t": pack_t(v2c).astype(np.float16),
                "v2n": pack_n(v2c).astype(np.float16),
                "v1n": pack_n(v1c).astype(bf16),
                "b2r": np.ascontiguousarray(b2rep),
                "bcol": bcol,
                "ones2": np.ones((128, 2), bf16),
                "cm": np.ascontiguousarray(np.concatenate([k1, k2], axis=1)),
                "idh": np.eye(128, dtype=np.float16),
            }
        )
    return in_maps


def kernel(v1, v2, v1_mask, v2_mask):
    global LAST_RESULTS
    from concourse.bass_utils import run_bass_kernel_spmd

    nc = get_nc()
    in_maps = _host_prep(v1, v2, v1_mask, v2_mask)
    res = run_bass_kernel_spmd(nc, in_maps, list(range(NCORES)))
    LAST_RESULTS = res

    def unpack(name):
        parts = []
        for k in range(NCORES):
            arr = res.results[k][name].astype(np.float32)
            parts.append(arr.reshape(EPC, 128, NB, D).transpose(0, 2, 1, 3).reshape(EPC, L, D))
        return np.concatenate(parts, axis=0)

    return unpack("o1"), unpack("o2")
